# revision 32
# baseline (speedup 1.0000x reference)
"""Causal self-attention (RoPE, 16 heads, B=2 T=2048 C=1024) on 8 TRN2 cores.

Sharding: core = b*4 + g  (b = batch, g = head-group of 4 heads).
Each core computes the qkv projection for its 4 heads, RoPE, causal flash
attention, and the w_proj partial product for its head slice; the host sums
the 4 partials per batch.

v2 design notes (from HW trace analysis):
- The PE clock gate (HAM) halves the PE clock unless it streams
  back-to-back; dependency stalls in the attention loop kept the whole
  attention phase at 1.2 GHz in v1.  v2 keeps the PE fed: qkv work is
  emitted as filler inside pair 0's attention loop and the output
  projection inside pair 1's, one unit per attention iteration.
- Scores use K=64 row-tiled matmul pairs (heads 2hp/2hp+1 on PE rows 0-63
  and 64-127) which execute concurrently on the PE sub-arrays.
- exp() alternates between the Scalar engine (exact table lookup) and the
  Vector engine (Schraudolph bit-trick straight into bf16 bits:
  int16(x*A + B), ~2% rel err) so neither engine paces the loop.
- Attention runs on 512-wide query slices so PSUM fits: score ring 4 banks
  + y ring 2 banks + qkv/proj accumulator ring 2 banks = 8 banks.
"""

import numpy as np

# Problem constants (hardcoded per harness contract).
B = 2
T = 2048
C = 1024
N_HEAD = 16
HD = 64
HPC = 4           # heads per core
N_CORES = 8
ROPE_BASE = 10000.0
TS = 512          # qkv t-slice width
TQ = 512          # attention query-slice width
VW = HD + 1       # v_ext per-head width (v + ones column for denominators)

DTCFG = "bbbb"    # kept for test.py compat; v2 is bf16-only

# Schraudolph exp -> bf16 bits: bf16(exp(x)) ~ int16(x * A + B)
SCH_A = 128.0 / float(np.log(2.0))
SCH_B = 16256.0 - 7.4

_CACHE = {}


def _np_dt(ch):
    if ch == "b":
        import ml_dtypes
        return np.dtype(ml_dtypes.bfloat16)
    return np.dtype(np.float32)


def _build(t_len=T, dtcfg=None, debug=False):
    import concourse.tile as tile
    import concourse.bass as bass
    from concourse import bacc, mybir

    F32 = mybir.dt.float32
    BF16 = mybir.dt.bfloat16
    I16 = mybir.dt.int16
    MULT = mybir.AluOpType.mult
    ADD = mybir.AluOpType.add
    EXP = mybir.ActivationFunctionType.Exp

    n_ts = t_len // TS          # qkv t-slices
    n_tt = t_len // 128         # 128-row t-tiles
    n_j = t_len // TQ           # attention query slices per head pair
    n_ipj = TQ // 128           # new key tiles per query slice

    nc = bacc.Bacc(None, target_bir_lowering=False, debug=False)
    with tile.TileContext(nc) as tc:
        with tc.tile_pool(name="dram", bufs=1, space="DRAM") as dram:
            xT = dram.tile([C, t_len], BF16, kind="ExternalInput")
            wqk = dram.tile([C, 8 * HD], BF16, kind="ExternalInput")
            wv = dram.tile([C, 4 * HD], BF16, kind="ExternalInput")
            wo = dram.tile([4 * HD, C], BF16, kind="ExternalInput")
            cost = dram.tile([128, t_len], F32, kind="ExternalInput")
            ssin = dram.tile([128, t_len], F32, kind="ExternalInput")
            utri = dram.tile([128, 128], BF16, kind="ExternalInput")
            out = dram.tile([t_len, C], F32, kind="ExternalOutput")

            xT_c = xT.rearrange("(a p) t -> a p t", p=128)    # [8, 128, T]
            wqk_c = wqk.rearrange("(a p) m -> a p m", p=128)  # [8, 128, 512]
            wv_c = wv.rearrange("(a p) m -> a p m", p=128)    # [8, 128, 256]
            wo_c = wo.rearrange("(a p) m -> a p m", p=128)    # [2, 128, 1024]

            with (
                tc.tile_pool(name="persist", bufs=1) as persist,
                tc.tile_pool(name="rope_pool", bufs=3) as rope_pool,
                tc.tile_pool(name="p_pool", bufs=6) as p_pool,
                tc.tile_pool(name="n_pool", bufs=3) as n_pool,
                tc.tile_pool(name="o_pool", bufs=4) as o_pool,
                tc.tile_pool(name="acc_ps", bufs=2, space="PSUM") as acc_ps,
                tc.tile_pool(name="s_ps", bufs=4, space="PSUM") as s_ps,
                tc.tile_pool(name="y_ps", bufs=2, space="PSUM") as y_ps,
            ):
                # ---- persistent SBUF ----
                utri_sb = persist.tile([128, 128], BF16)
                cos_sb = persist.tile([128, t_len], F32)
                ssin_sb = persist.tile([128, t_len], F32)
                wqk_sb = [persist.tile([128, 8 * HD], BF16, name=f"wqk{c}")
                          for c in range(8)]
                wv_sb = [persist.tile([128, 4 * HD], BF16, name=f"wv{c}")
                         for c in range(8)]
                wo_sb = [persist.tile([128, C], BF16, name=f"wo{k}")
                         for k in range(2)]
                xT_sb = [persist.tile([128, t_len], BF16, name=f"xTsb{c}")
                         for c in range(8)]
                qkT = [persist.tile([128, t_len], BF16, name=f"qkT{m}")
                       for m in range(4)]
                vext_sb = persist.tile([128, n_tt * HPC * VW], BF16)
                vext_v = vext_sb.rearrange("p (i h d) -> p i h d",
                                           i=n_tt, d=VW)
                yT = [persist.tile([128, t_len], BF16, name=f"yT{k}")
                      for k in range(2)]

                # ones columns of v_ext (memset; v copies overwrite cols 0-63)
                nc.gpsimd.memset(vext_sb, 1.0)

                # ---- input DMAs ----
                # weights first (small, needed by the first matmuls), then x
                # and the RoPE tables in 512-column waves across 4 queues so
                # wave 0 lands in ~2us and compute starts immediately.  The
                # scalar queue carries no DMA (it is the exp engine).
                qs = [nc.sync, nc.gpsimd, nc.scalar]
                for c in range(8):
                    qs[c % 3].dma_start(out=wv_sb[c], in_=wv_c[c])
                for c in range(8):
                    qs[c % 3].dma_start(out=wqk_sb[c], in_=wqk_c[c])
                nc.gpsimd.dma_start(out=utri_sb, in_=utri[:])
                qi = 0
                for w in range(n_ts):
                    lo, hi = w * TS, (w + 1) * TS
                    for c in range(8):
                        qs[qi % 3].dma_start(out=xT_sb[c][:, lo:hi],
                                             in_=xT_c[c, :, lo:hi])
                        qi += 1
                    qs[qi % 3].dma_start(out=cos_sb[:, lo:hi],
                                         in_=cost[:, lo:hi])
                    qi += 1
                    qs[qi % 3].dma_start(out=ssin_sb[:, lo:hi],
                                         in_=ssin[:, lo:hi])
                    qi += 1
                for k in range(2):
                    nc.sync.dma_start(out=wo_sb[k], in_=wo_c[k])

                # ---- emission units ----
                def emit_v_unit(tt):
                    """v projection for t-tiles tt, tt+1 -> vext."""
                    vps = acc_ps.tile([128, 512], F32, tag="acc",
                                      name=f"vps_{tt}")
                    for b2 in range(2):
                        t0 = (tt + b2) * 128
                        for c in range(8):
                            nc.tensor.matmul(
                                out=vps[:, b2 * 256:(b2 + 1) * 256],
                                lhsT=xT_sb[c][:, t0:t0 + 128],
                                rhs=wv_sb[c][:],
                                start=(c == 0), stop=(c == 7),
                            )
                    nc.scalar.copy(
                        out=vext_v[:, tt:tt + 2, :, :HD],
                        in_=vps.rearrange("p (b h d) -> p b h d", b=2, d=HD),
                    )

                def emit_qk_unit(m, ts):
                    """q/k projection group (m, t-slice ts) + RoPE."""
                    t0 = ts * TS
                    qkps = acc_ps.tile([128, TS], F32, tag="acc",
                                       name=f"qkps_{m}_{ts}")
                    for c in range(8):
                        nc.tensor.matmul(
                            out=qkps[:],
                            lhsT=wqk_sb[c][:, m * 128:(m + 1) * 128],
                            rhs=xT_sb[c][:, t0:t0 + TS],
                            start=(c == 0), stop=(c == 7),
                        )
                    # RoPE: qkT = qkps*cos + blockswap32(qkps*ssin_preswap)
                    u = rope_pool.tile([128, TS], BF16, tag="u",
                                       name=f"u_{m}_{ts}")
                    nc.vector.tensor_mul(u, qkps[:], ssin_sb[:, t0:t0 + TS])
                    u2 = rope_pool.tile([128, TS], BF16, tag="u2",
                                        name=f"u2_{m}_{ts}")
                    for hb in (0, 64):
                        nc.sync.dma_start(out=u2[hb:hb + 32, :],
                                          in_=u[hb + 32:hb + 64, :])
                        nc.sync.dma_start(out=u2[hb + 32:hb + 64, :],
                                          in_=u[hb:hb + 32, :])
                    t1 = rope_pool.tile([128, TS], BF16, tag="t1",
                                        name=f"t1_{m}_{ts}")
                    nc.vector.tensor_mul(t1, qkps[:], cos_sb[:, t0:t0 + TS])
                    nc.gpsimd.tensor_add(qkT[m][:, t0:t0 + TS], t1, u2)

                def emit_proj_unit(tt):
                    """output projection + store for t-tile tt."""
                    ops = [acc_ps.tile([128, 512], F32, tag="acc",
                                       name=f"ops_{tt}_{cs}")
                           for cs in range(2)]
                    for k in range(2):
                        for cs in range(2):
                            nc.tensor.matmul(
                                out=ops[cs][:],
                                lhsT=yT[k][:, tt * 128:(tt + 1) * 128],
                                rhs=wo_sb[k][:, cs * 512:(cs + 1) * 512],
                                start=(k == 0), stop=(k == 1),
                            )
                    for cs in range(2):
                        osb = o_pool.tile([128, 512], F32, tag="osb",
                                          name=f"osb_{tt}_{cs}")
                        if cs == 0:
                            nc.scalar.copy(out=osb, in_=ops[cs][:])
                        else:
                            nc.vector.tensor_copy(out=osb, in_=ops[cs][:])
                        (nc.sync if cs == 0 else nc.gpsimd).dma_start(
                            out=out[tt * 128:(tt + 1) * 128,
                                    cs * 512:(cs + 1) * 512],
                            in_=osb,
                        )

                # ---- attention for one head pair ----
                def attention_pair(hp, filler_by_slice):
                    """Heads (2hp, 2hp+1); consumes one PE filler unit per
                    iteration (per slice j from filler_by_slice[j])."""
                    qtile, ktile = qkT[hp], qkT[2 + hp]
                    heads = (0, 1)

                    for j in range(n_j):
                        filler = list(filler_by_slice.get(j, []))
                        base = TQ * j
                        n_i = n_ipj * j + n_ipj
                        # spread filler units evenly over the slice
                        fire = set()
                        if filler:
                            step = max(1, n_i // len(filler))
                            fire = {k * step for k in range(len(filler))}
                        yps = {hh: y_ps.tile([VW, TQ], F32, tag="yps",
                                             name=f"yps_{hp}_{hh}_{j}")
                               for hh in heads}
                        pend = []

                        def emit_s(i):
                            if filler and i in fire:
                                filler.pop(0)()
                            off = max(base, 128 * i) - base
                            sx = {}
                            for hh in heads:
                                sx[hh] = s_ps.tile([128, TQ], F32, tag="sps",
                                                   name=f"s_{hp}_{hh}_{j}_{i}")
                            for hh in heads:
                                hoff = 64 * hh
                                nc.tensor.matmul(
                                    out=sx[hh][:, off:],
                                    lhsT=ktile[hoff:hoff + 64,
                                               128 * i:128 * (i + 1)],
                                    rhs=qtile[hoff:hoff + 64,
                                              base + off:base + TQ],
                                    start=True, stop=True,
                                )
                            px = p_pool.tile([128, 2 * TQ], BF16, tag="px",
                                             name=f"px_{hp}_{j}_{i}")
                            for hh in heads:
                                hcol = TQ * hh
                                if (i + hh) % 2 == 0:
                                    nc.scalar.activation(
                                        out=px[:, hcol + off:hcol + TQ],
                                        in_=sx[hh][:, off:], func=EXP)
                                else:
                                    e16 = px.bitcast(I16)
                                    nc.vector.tensor_scalar(
                                        out=e16[:, hcol + off:hcol + TQ],
                                        in0=sx[hh][:, off:],
                                        scalar1=SCH_A, scalar2=SCH_B,
                                        op0=MULT, op1=ADD)
                            if i >= n_ipj * j:
                                for hh in heads:
                                    hcol = TQ * hh
                                    nc.vector.tensor_mul(
                                        px[:, hcol + off:hcol + off + 128],
                                        px[:, hcol + off:hcol + off + 128],
                                        utri_sb)
                            pend.append((i, px))

                        def emit_y():
                            i, px = pend.pop(0)
                            off = max(base, 128 * i) - base
                            for hh in heads:
                                gh = 2 * hp + hh
                                base_v = (i * HPC + gh) * VW
                                nc.tensor.matmul(
                                    out=yps[hh][:, off:],
                                    lhsT=vext_sb[:, base_v:base_v + VW],
                                    rhs=px[:, TQ * hh + off:TQ * (hh + 1)],
                                    start=(i == 0), stop=(i == n_i - 1),
                                    skip_group_check=True,
                                )

                        for i in range(n_i):
                            emit_s(i)
                            if i >= 2:
                                emit_y()
                        emit_y()
                        emit_y()
                        for f in filler:   # leftovers (shouldn't happen)
                            f()

                        # normalization: y /= softmax denominator (row 64)
                        ycp = n_pool.tile([VW, 2 * TQ], F32, tag="ycp",
                                          name=f"ycp_{hp}_{j}")
                        nc.scalar.copy(out=ycp[:, 0:TQ], in_=yps[0][:])
                        nc.scalar.copy(out=ycp[:, TQ:], in_=yps[1][:])
                        strip = n_pool.tile([8, 128], F32, tag="strip",
                                            name=f"strip_{hp}_{j}")
                        nc.sync.dma_start(
                            out=strip,
                            in_=ycp[HD:HD + 1, :].rearrange(
                                "p (a b) -> p a b", b=128))
                        rstrip = n_pool.tile([8, 128], F32, tag="rstrip",
                                             name=f"rstrip_{hp}_{j}")
                        nc.vector.reciprocal_approx_fast(out=rstrip,
                                                         in_=strip)
                        rrow = n_pool.tile([1, 2 * TQ], F32, tag="rrow",
                                           name=f"rrow_{hp}_{j}")
                        nc.sync.dma_start(
                            out=rrow.rearrange("p (a b) -> p a b", b=128),
                            in_=rstrip)
                        bcast = n_pool.tile([64, 2 * TQ], F32, tag="bcast",
                                            name=f"bcast_{hp}_{j}")
                        nc.gpsimd.partition_broadcast(bcast[:], rrow[:])
                        for hh in heads:
                            nc.vector.tensor_mul(
                                yT[hp][64 * hh:64 * hh + 64,
                                       base:base + TQ],
                                ycp[0:HD, TQ * hh:TQ * hh + TQ],
                                bcast[:, TQ * hh:TQ * hh + TQ])

                # ---- schedule ----
                # upfront: v tiles 0-3, q/k pair 0 t-slice 0
                emit_v_unit(0)
                emit_v_unit(2)
                emit_qk_unit(2, 0)   # k01 ts0
                emit_qk_unit(0, 0)   # q01 ts0

                # pair 0 attention; filler = rest of qkv, scheduled so each
                # slice's q/k and v dependencies are emitted a slice ahead
                fill0 = {
                    0: [lambda: emit_qk_unit(2, 1),
                        lambda: emit_qk_unit(0, 1),
                        lambda: emit_v_unit(4)],
                    1: [lambda: emit_v_unit(6),
                        lambda: emit_qk_unit(2, 2),
                        lambda: emit_qk_unit(0, 2),
                        lambda: emit_v_unit(8),
                        lambda: emit_v_unit(10)],
                    2: [lambda: emit_qk_unit(2, 3),
                        lambda: emit_qk_unit(0, 3),
                        lambda: emit_v_unit(12),
                        lambda: emit_v_unit(14),
                        lambda: emit_qk_unit(3, 0),
                        lambda: emit_qk_unit(1, 0)],
                    3: [lambda: emit_qk_unit(3, 1),
                        lambda: emit_qk_unit(1, 1),
                        lambda: emit_qk_unit(3, 2),
                        lambda: emit_qk_unit(1, 2),
                        lambda: emit_qk_unit(3, 3),
                        lambda: emit_qk_unit(1, 3)],
                }
                attention_pair(0, fill0)

                # pair 1 attention; filler = output projection for t-tiles
                # whose yT rows are complete (tile tt needs pair-1 slice
                # tt//n_ipj normed; norms for slice j are emitted at the
                # end of slice j)
                fill1 = {
                    1: [lambda tt=tt: emit_proj_unit(tt) for tt in range(2)],
                    2: [lambda tt=tt: emit_proj_unit(tt)
                        for tt in range(2, 6)],
                    3: [lambda tt=tt: emit_proj_unit(tt)
                        for tt in range(6, 12)],
                }
                attention_pair(1, fill1)

                # tail: remaining projection tiles
                for tt in range(12, n_tt):
                    emit_proj_unit(tt)

    nc.compile()
    names = dict(
        xT=xT.name, wqk=wqk.name, wv=wv.name, wo=wo.name,
        cost=cost.name, ssin=ssin.name, utri=utri.name,
        out=out.name,
    )
    return nc, names


# Head-dim permutation: evens first, odds last — turns the interleaved
# rotate-half pair swap into a contiguous 32-row block swap on device.
PERM = np.concatenate([np.arange(0, HD, 2), np.arange(1, HD, 2)])


def _host_constants(t_len=T, dtcfg=None):
    inv_freq = 1.0 / (ROPE_BASE ** (np.arange(0, HD, 2, dtype=np.float64) / HD))
    t = np.arange(t_len, dtype=np.float64)
    freqs = np.outer(t, inv_freq)                      # [T, 32]
    emb = np.concatenate([freqs, freqs], axis=-1)      # [T, 64]
    cosT = np.cos(emb).T.astype(np.float32)            # [64, T]
    sinT = np.sin(emb).T.astype(np.float32)
    sgn = np.where(np.arange(HD) % 2 == 0, -1.0, 1.0).astype(np.float32)
    ssinT = sinT * sgn[:, None]
    cosP, ssinP = cosT[PERM], ssinT[PERM]
    cos128 = np.vstack([cosP, cosP]).copy()            # [128, T]
    ssin128 = np.vstack([ssinP, ssinP])
    # pre-swap the sin table in 32-blocks: on device the product
    # u = q*ssin_preswap is block-swapped, giving swap(q)*ssin
    sw = np.concatenate([np.arange(32, 64), np.arange(0, 32),
                         np.arange(96, 128), np.arange(64, 96)])
    ssin128 = ssin128[sw].copy()
    utri = np.triu(np.ones((128, 128), dtype=np.float32)).astype(
        _np_dt("b"))
    return cos128, ssin128, utri


def _perm_heads(w):
    """Permute each head's 64 columns of w [C, HPC*HD] by PERM."""
    Cdim = w.shape[0]
    return w.reshape(Cdim, HPC, HD)[:, :, PERM].reshape(Cdim, HPC * HD)


def _core_inputs(x, w_attn, w_proj, t_len=T, dtcfg=None):
    """Build the per-core input maps (values only, keyed by logical name)."""
    d_b = _np_dt("b")
    cos128, ssin128, utri = _host_constants(t_len, dtcfg)
    per_core = []
    for core in range(N_CORES):
        b, g = divmod(core, 4)
        h0 = g * HPC * HD                       # column offset of first head
        wq = _perm_heads(w_attn[:, h0:h0 + HPC * HD])
        wk = _perm_heads(w_attn[:, C + h0:C + h0 + HPC * HD]
                         * np.float32(1.0 / np.sqrt(HD)))
        wvs = w_attn[:, 2 * C + h0:2 * C + h0 + HPC * HD]
        per_core.append(dict(
            xT=np.ascontiguousarray(x[b].T).astype(d_b),
            wqk=np.ascontiguousarray(np.concatenate([wq, wk], axis=1)).astype(d_b),
            wv=np.ascontiguousarray(wvs).astype(d_b),
            wo=np.ascontiguousarray(w_proj[h0:h0 + HPC * HD, :]).astype(d_b),
            cost=cos128, ssin=ssin128, utri=utri,
        ))
    return per_core


def kernel(x, w_attn, w_proj):
    from concourse.bass_utils import run_bass_kernel_spmd

    x = np.asarray(x, dtype=np.float32)
    w_attn = np.asarray(w_attn, dtype=np.float32)
    w_proj = np.asarray(w_proj, dtype=np.float32)

    if "nc" not in _CACHE:
        _CACHE["nc"], _CACHE["names"] = _build(T)
    nc, names = _CACHE["nc"], _CACHE["names"]

    per_core = _core_inputs(x, w_attn, w_proj, T)
    in_maps = [{names[k]: v for k, v in m.items()} for m in per_core]
    r = run_bass_kernel_spmd(nc, in_maps, core_ids=list(range(N_CORES)))

    full = np.zeros((B, T, C), dtype=np.float64)
    for core in range(N_CORES):
        full[core // 4] += r.results[core][names["out"]].astype(np.float64)
    return full.astype(np.float32)


# revision 38
# speedup vs baseline: 1.0146x; 1.0146x over previous
"""Causal self-attention (RoPE, 16 heads, B=2 T=2048 C=1024) on 8 TRN2 cores.

Sharding: core = b*4 + g  (b = batch, g = head-group of 4 heads).
Each core computes the qkv projection for its 4 heads, RoPE, causal flash
attention, and the w_proj partial product for its head slice; the host sums
the 4 partials per batch.

v2 design notes (from HW trace analysis):
- The PE clock gate (HAM) halves the PE clock unless it streams
  back-to-back; dependency stalls in the attention loop kept the whole
  attention phase at 1.2 GHz in v1.  v2 keeps the PE fed: qkv work is
  emitted as filler inside pair 0's attention loop and the output
  projection inside pair 1's, one unit per attention iteration.
- Scores use K=64 row-tiled matmul pairs (heads 2hp/2hp+1 on PE rows 0-63
  and 64-127) which execute concurrently on the PE sub-arrays.
- exp() alternates between the Scalar engine (exact table lookup) and the
  Vector engine (Schraudolph bit-trick straight into bf16 bits:
  int16(x*A + B), ~2% rel err) so neither engine paces the loop.
- Attention runs on 512-wide query slices so PSUM fits: score ring 4 banks
  + y ring 2 banks + qkv/proj accumulator ring 2 banks = 8 banks.
"""

import numpy as np

# Problem constants (hardcoded per harness contract).
B = 2
T = 2048
C = 1024
N_HEAD = 16
HD = 64
HPC = 4           # heads per core
N_CORES = 8
ROPE_BASE = 10000.0
TS = 512          # qkv t-slice width
TQ = 512          # attention query-slice width
VW = HD + 1       # v_ext per-head width (v + ones column for denominators)

DTCFG = "bbbb"    # kept for test.py compat; v2 is bf16-only

# Schraudolph exp -> bf16 bits: bf16(exp(x)) ~ int16(x * A + B)
SCH_A = 128.0 / float(np.log(2.0))
SCH_B = 16256.0 - 7.4

_CACHE = {}


def _np_dt(ch):
    if ch == "b":
        import ml_dtypes
        return np.dtype(ml_dtypes.bfloat16)
    return np.dtype(np.float32)


def _build(t_len=T, dtcfg=None, debug=False):
    import concourse.tile as tile
    import concourse.bass as bass
    from concourse import bacc, mybir

    F32 = mybir.dt.float32
    BF16 = mybir.dt.bfloat16
    I16 = mybir.dt.int16
    MULT = mybir.AluOpType.mult
    ADD = mybir.AluOpType.add
    EXP = mybir.ActivationFunctionType.Exp

    n_ts = t_len // TS          # qkv t-slices
    n_tt = t_len // 128         # 128-row t-tiles
    n_j = t_len // TQ           # attention query slices per head pair
    n_ipj = TQ // 128           # new key tiles per query slice

    nc = bacc.Bacc(None, target_bir_lowering=False, debug=False)
    with tile.TileContext(nc) as tc:
        with tc.tile_pool(name="dram", bufs=1, space="DRAM") as dram:
            xT = dram.tile([C, t_len], BF16, kind="ExternalInput")
            wqk = dram.tile([C, 8 * HD], BF16, kind="ExternalInput")
            wv = dram.tile([C, 4 * HD], BF16, kind="ExternalInput")
            wo = dram.tile([4 * HD, C], BF16, kind="ExternalInput")
            cost = dram.tile([128, t_len], BF16, kind="ExternalInput")
            ssin = dram.tile([128, t_len], BF16, kind="ExternalInput")
            utri = dram.tile([128, 128], BF16, kind="ExternalInput")
            out = dram.tile([t_len, C], F32, kind="ExternalOutput")

            xT_c = xT.rearrange("(a p) t -> a p t", p=128)    # [8, 128, T]
            wqk_c = wqk.rearrange("(a p) m -> a p m", p=128)  # [8, 128, 512]
            wv_c = wv.rearrange("(a p) m -> a p m", p=128)    # [8, 128, 256]
            wo_c = wo.rearrange("(a p) m -> a p m", p=128)    # [2, 128, 1024]

            with (
                tc.tile_pool(name="persist", bufs=1) as persist,
                tc.tile_pool(name="rope_pool", bufs=3) as rope_pool,
                tc.tile_pool(name="p_pool", bufs=6) as p_pool,
                tc.tile_pool(name="n_pool", bufs=3) as n_pool,
                tc.tile_pool(name="o_pool", bufs=4) as o_pool,
                tc.tile_pool(name="acc_ps", bufs=2, space="PSUM") as acc_ps,
                tc.tile_pool(name="s_ps", bufs=4, space="PSUM") as s_ps,
                tc.tile_pool(name="y_ps", bufs=2, space="PSUM") as y_ps,
            ):
                # ---- persistent SBUF ----
                utri_sb = persist.tile([128, 128], BF16)
                cos_sb = persist.tile([128, t_len], BF16)
                ssin_sb = persist.tile([128, t_len], BF16)
                wqk_sb = [persist.tile([128, 8 * HD], BF16, name=f"wqk{c}")
                          for c in range(8)]
                wv_sb = [persist.tile([128, 4 * HD], BF16, name=f"wv{c}")
                         for c in range(8)]
                wo_sb = [persist.tile([128, C], BF16, name=f"wo{k}")
                         for k in range(2)]
                xT_sb = [persist.tile([128, t_len], BF16, name=f"xTsb{c}")
                         for c in range(8)]
                qkT = [persist.tile([128, t_len], BF16, name=f"qkT{m}")
                       for m in range(4)]
                vext_sb = persist.tile([128, n_tt * HPC * VW], BF16)
                vext_v = vext_sb.rearrange("p (i h d) -> p i h d",
                                           i=n_tt, d=VW)
                yT = [persist.tile([128, t_len], BF16, name=f"yT{k}")
                      for k in range(2)]

                # ones columns of v_ext (memset; v copies overwrite cols 0-63)
                nc.gpsimd.memset(vext_sb, 1.0)

                # ---- input DMAs ----
                # weights first (small, needed by the first matmuls), then x
                # and the RoPE tables in 512-column waves across 4 queues so
                # wave 0 lands in ~2us and compute starts immediately.  The
                # scalar queue carries no DMA (it is the exp engine).
                qs = [nc.sync, nc.gpsimd, nc.scalar]
                # order: wv + x-wave0 (unblocks the v matmuls at ~4us),
                # then RoPE tables + wqk (first q/k group), then the rest
                # of x as one full-width descriptor per chunk
                for c in range(8):
                    qs[c % 3].dma_start(out=wv_sb[c], in_=wv_c[c])
                for c in range(8):
                    qs[c % 3].dma_start(out=xT_sb[c][:, 0:TS],
                                        in_=xT_c[c, :, 0:TS])
                nc.sync.dma_start(out=cos_sb[:, 0:TS], in_=cost[:, 0:TS])
                nc.gpsimd.dma_start(out=ssin_sb[:, 0:TS],
                                    in_=ssin[:, 0:TS])
                for c in range(8):
                    qs[c % 3].dma_start(out=wqk_sb[c], in_=wqk_c[c])
                nc.scalar.dma_start(out=utri_sb, in_=utri[:])
                nc.sync.dma_start(out=cos_sb[:, TS:], in_=cost[:, TS:])
                nc.gpsimd.dma_start(out=ssin_sb[:, TS:], in_=ssin[:, TS:])
                for c in range(8):
                    qs[c % 3].dma_start(out=xT_sb[c][:, TS:],
                                        in_=xT_c[c, :, TS:])
                for k in range(2):
                    qs[k].dma_start(out=wo_sb[k], in_=wo_c[k])

                # ---- emission units ----
                def emit_v_unit(tt):
                    """v projection for t-tiles tt, tt+1 -> vext."""
                    vps = acc_ps.tile([128, 512], F32, tag="acc",
                                      name=f"vps_{tt}")
                    for b2 in range(2):
                        t0 = (tt + b2) * 128
                        for c in range(8):
                            nc.tensor.matmul(
                                out=vps[:, b2 * 256:(b2 + 1) * 256],
                                lhsT=xT_sb[c][:, t0:t0 + 128],
                                rhs=wv_sb[c][:],
                                start=(c == 0), stop=(c == 7),
                            )
                    nc.scalar.copy(
                        out=vext_v[:, tt:tt + 2, :, :HD],
                        in_=vps.rearrange("p (b h d) -> p b h d", b=2, d=HD),
                    )

                def emit_qk_unit(m, ts):
                    """q/k projection group (m, t-slice ts) + RoPE."""
                    t0 = ts * TS
                    qkps = acc_ps.tile([128, TS], F32, tag="acc",
                                       name=f"qkps_{m}_{ts}")
                    for c in range(8):
                        nc.tensor.matmul(
                            out=qkps[:],
                            lhsT=wqk_sb[c][:, m * 128:(m + 1) * 128],
                            rhs=xT_sb[c][:, t0:t0 + TS],
                            start=(c == 0), stop=(c == 7),
                        )
                    # RoPE: qkT = qkps*cos + blockswap32(qkps*ssin_preswap)
                    u = rope_pool.tile([128, TS], BF16, tag="u",
                                       name=f"u_{m}_{ts}")
                    nc.vector.tensor_mul(u, qkps[:], ssin_sb[:, t0:t0 + TS])
                    u2 = rope_pool.tile([128, TS], BF16, tag="u2",
                                        name=f"u2_{m}_{ts}")
                    for hb in (0, 64):
                        nc.sync.dma_start(out=u2[hb:hb + 32, :],
                                          in_=u[hb + 32:hb + 64, :])
                        nc.sync.dma_start(out=u2[hb + 32:hb + 64, :],
                                          in_=u[hb:hb + 32, :])
                    t1 = rope_pool.tile([128, TS], BF16, tag="t1",
                                        name=f"t1_{m}_{ts}")
                    nc.vector.tensor_mul(t1, qkps[:], cos_sb[:, t0:t0 + TS])
                    nc.gpsimd.tensor_add(qkT[m][:, t0:t0 + TS], t1, u2)

                def emit_proj_unit(tt):
                    """output projection + store for t-tile tt."""
                    ops = [acc_ps.tile([128, 512], F32, tag="acc",
                                       name=f"ops_{tt}_{cs}")
                           for cs in range(2)]
                    for k in range(2):
                        for cs in range(2):
                            nc.tensor.matmul(
                                out=ops[cs][:],
                                lhsT=yT[k][:, tt * 128:(tt + 1) * 128],
                                rhs=wo_sb[k][:, cs * 512:(cs + 1) * 512],
                                start=(k == 0), stop=(k == 1),
                            )
                    for cs in range(2):
                        osb = o_pool.tile([128, 512], F32, tag="osb",
                                          name=f"osb_{tt}_{cs}")
                        if cs == 0:
                            nc.scalar.copy(out=osb, in_=ops[cs][:])
                        else:
                            nc.vector.tensor_copy(out=osb, in_=ops[cs][:])
                        (nc.sync if cs == 0 else nc.gpsimd).dma_start(
                            out=out[tt * 128:(tt + 1) * 128,
                                    cs * 512:(cs + 1) * 512],
                            in_=osb,
                        )

                # ---- attention for one head pair ----
                def attention_pair(hp, filler_by_slice):
                    """Heads (2hp, 2hp+1); consumes one PE filler unit per
                    iteration (per slice j from filler_by_slice[j])."""
                    qtile, ktile = qkT[hp], qkT[2 + hp]
                    heads = (0, 1)

                    for j in range(n_j):
                        filler = list(filler_by_slice.get(j, []))
                        base = TQ * j
                        n_i = n_ipj * j + n_ipj
                        # burst 2 filler units/iter at the slice start (covers
                        # the PE while the previous slice's exp backlog
                        # drains), then spread the rest over the slice
                        fire = {}
                        if filler:
                            nb = min(len(filler), 4)
                            fire = {0: 2, 1: 2} if nb >= 4 else {0: nb}
                            rest = len(filler) - sum(fire.values())
                            if rest > 0:
                                step = max(1, (n_i - 2) // rest)
                                for k in range(rest):
                                    it = min(2 + k * step, n_i - 1)
                                    fire[it] = fire.get(it, 0) + 1
                        yps = {hh: y_ps.tile([VW, TQ], F32, tag="yps",
                                             name=f"yps_{hp}_{hh}_{j}")
                               for hh in heads}
                        pend = []

                        def emit_s(i):
                            for _ in range(fire.get(i, 0)):
                                if filler:
                                    filler.pop(0)()
                            off = max(base, 128 * i) - base
                            sx = {}
                            for hh in heads:
                                sx[hh] = s_ps.tile([128, TQ], F32, tag="sps",
                                                   name=f"s_{hp}_{hh}_{j}_{i}")
                            for hh in heads:
                                hoff = 64 * hh
                                nc.tensor.matmul(
                                    out=sx[hh][:, off:],
                                    lhsT=ktile[hoff:hoff + 64,
                                               128 * i:128 * (i + 1)],
                                    rhs=qtile[hoff:hoff + 64,
                                              base + off:base + TQ],
                                    start=True, stop=True,
                                )
                            px = p_pool.tile([128, 2 * TQ], BF16, tag="px",
                                             name=f"px_{hp}_{j}_{i}")
                            for hh in heads:
                                hcol = TQ * hh
                                if (i + hh) % 2 == 0:
                                    nc.scalar.activation(
                                        out=px[:, hcol + off:hcol + TQ],
                                        in_=sx[hh][:, off:], func=EXP)
                                else:
                                    e16 = px.bitcast(I16)
                                    nc.vector.tensor_scalar(
                                        out=e16[:, hcol + off:hcol + TQ],
                                        in0=sx[hh][:, off:],
                                        scalar1=SCH_A, scalar2=SCH_B,
                                        op0=MULT, op1=ADD)
                            if i >= n_ipj * j:
                                for hh in heads:
                                    hcol = TQ * hh
                                    nc.vector.tensor_mul(
                                        px[:, hcol + off:hcol + off + 128],
                                        px[:, hcol + off:hcol + off + 128],
                                        utri_sb)
                            pend.append((i, px))

                        def emit_y():
                            i, px = pend.pop(0)
                            off = max(base, 128 * i) - base
                            for hh in heads:
                                gh = 2 * hp + hh
                                base_v = (i * HPC + gh) * VW
                                nc.tensor.matmul(
                                    out=yps[hh][:, off:],
                                    lhsT=vext_sb[:, base_v:base_v + VW],
                                    rhs=px[:, TQ * hh + off:TQ * (hh + 1)],
                                    start=(i == 0), stop=(i == n_i - 1),
                                    skip_group_check=True,
                                )

                        for i in range(n_i):
                            emit_s(i)
                            if i >= 2:
                                emit_y()
                        emit_y()
                        emit_y()
                        for f in filler:   # leftovers (shouldn't happen)
                            f()

                        # normalization: y /= softmax denominator (row 64)
                        ycp = n_pool.tile([VW, 2 * TQ], F32, tag="ycp",
                                          name=f"ycp_{hp}_{j}")
                        nc.scalar.copy(out=ycp[:, 0:TQ], in_=yps[0][:])
                        nc.scalar.copy(out=ycp[:, TQ:], in_=yps[1][:])
                        strip = n_pool.tile([8, 128], F32, tag="strip",
                                            name=f"strip_{hp}_{j}")
                        nc.sync.dma_start(
                            out=strip,
                            in_=ycp[HD:HD + 1, :].rearrange(
                                "p (a b) -> p a b", b=128))
                        rstrip = n_pool.tile([8, 128], F32, tag="rstrip",
                                             name=f"rstrip_{hp}_{j}")
                        nc.vector.reciprocal_approx_fast(out=rstrip,
                                                         in_=strip)
                        rrow = n_pool.tile([1, 2 * TQ], F32, tag="rrow",
                                           name=f"rrow_{hp}_{j}")
                        nc.sync.dma_start(
                            out=rrow.rearrange("p (a b) -> p a b", b=128),
                            in_=rstrip)
                        bcast = n_pool.tile([64, 2 * TQ], F32, tag="bcast",
                                            name=f"bcast_{hp}_{j}")
                        nc.gpsimd.partition_broadcast(bcast[:], rrow[:])
                        for hh in heads:
                            nc.vector.tensor_mul(
                                yT[hp][64 * hh:64 * hh + 64,
                                       base:base + TQ],
                                ycp[0:HD, TQ * hh:TQ * hh + TQ],
                                bcast[:, TQ * hh:TQ * hh + TQ])

                # ---- schedule ----
                # upfront: v tiles 0-3, q/k pair 0 t-slice 0
                emit_v_unit(0)
                emit_v_unit(2)
                emit_qk_unit(2, 0)   # k01 ts0
                emit_qk_unit(0, 0)   # q01 ts0

                # pair 0 attention; filler = rest of qkv, scheduled so each
                # slice's q/k and v dependencies are emitted a slice ahead
                fill0 = {
                    0: [lambda: emit_qk_unit(2, 1),
                        lambda: emit_qk_unit(0, 1),
                        lambda: emit_v_unit(4)],
                    1: [lambda: emit_v_unit(6),
                        lambda: emit_qk_unit(2, 2),
                        lambda: emit_qk_unit(0, 2),
                        lambda: emit_v_unit(8),
                        lambda: emit_v_unit(10)],
                    2: [lambda: emit_qk_unit(2, 3),
                        lambda: emit_qk_unit(0, 3),
                        lambda: emit_v_unit(12),
                        lambda: emit_v_unit(14),
                        lambda: emit_qk_unit(3, 0),
                        lambda: emit_qk_unit(1, 0)],
                    3: [lambda: emit_qk_unit(3, 1),
                        lambda: emit_qk_unit(1, 1),
                        lambda: emit_qk_unit(3, 2),
                        lambda: emit_qk_unit(1, 2),
                        lambda: emit_qk_unit(3, 3),
                        lambda: emit_qk_unit(1, 3)],
                }
                attention_pair(0, fill0)

                # pair 1 attention; filler = output projection for t-tiles
                # whose yT rows are complete (tile tt needs pair-1 slice
                # tt//n_ipj normed; norms for slice j are emitted at the
                # end of slice j)
                fill1 = {
                    1: [lambda tt=tt: emit_proj_unit(tt) for tt in range(2)],
                    2: [lambda tt=tt: emit_proj_unit(tt)
                        for tt in range(2, 6)],
                    3: [lambda tt=tt: emit_proj_unit(tt)
                        for tt in range(6, 12)],
                }
                attention_pair(1, fill1)

                # tail: remaining projection tiles
                for tt in range(12, n_tt):
                    emit_proj_unit(tt)

    nc.compile()
    names = dict(
        xT=xT.name, wqk=wqk.name, wv=wv.name, wo=wo.name,
        cost=cost.name, ssin=ssin.name, utri=utri.name,
        out=out.name,
    )
    return nc, names


# Head-dim permutation: evens first, odds last — turns the interleaved
# rotate-half pair swap into a contiguous 32-row block swap on device.
PERM = np.concatenate([np.arange(0, HD, 2), np.arange(1, HD, 2)])


def _host_constants(t_len=T, dtcfg=None):
    inv_freq = 1.0 / (ROPE_BASE ** (np.arange(0, HD, 2, dtype=np.float64) / HD))
    t = np.arange(t_len, dtype=np.float64)
    freqs = np.outer(t, inv_freq)                      # [T, 32]
    emb = np.concatenate([freqs, freqs], axis=-1)      # [T, 64]
    cosT = np.cos(emb).T.astype(np.float32)            # [64, T]
    sinT = np.sin(emb).T.astype(np.float32)
    sgn = np.where(np.arange(HD) % 2 == 0, -1.0, 1.0).astype(np.float32)
    ssinT = sinT * sgn[:, None]
    cosP, ssinP = cosT[PERM], ssinT[PERM]
    d_b = _np_dt("b")
    cos128 = np.vstack([cosP, cosP]).astype(d_b)       # [128, T]
    ssin128 = np.vstack([ssinP, ssinP])
    # pre-swap the sin table in 32-blocks: on device the product
    # u = q*ssin_preswap is block-swapped, giving swap(q)*ssin
    sw = np.concatenate([np.arange(32, 64), np.arange(0, 32),
                         np.arange(96, 128), np.arange(64, 96)])
    ssin128 = ssin128[sw].astype(d_b)
    utri = np.triu(np.ones((128, 128), dtype=np.float32)).astype(d_b)
    return cos128, ssin128, utri


def _perm_heads(w):
    """Permute each head's 64 columns of w [C, HPC*HD] by PERM."""
    Cdim = w.shape[0]
    return w.reshape(Cdim, HPC, HD)[:, :, PERM].reshape(Cdim, HPC * HD)


def _core_inputs(x, w_attn, w_proj, t_len=T, dtcfg=None):
    """Build the per-core input maps (values only, keyed by logical name)."""
    d_b = _np_dt("b")
    cos128, ssin128, utri = _host_constants(t_len, dtcfg)
    per_core = []
    for core in range(N_CORES):
        b, g = divmod(core, 4)
        h0 = g * HPC * HD                       # column offset of first head
        wq = _perm_heads(w_attn[:, h0:h0 + HPC * HD])
        wk = _perm_heads(w_attn[:, C + h0:C + h0 + HPC * HD]
                         * np.float32(1.0 / np.sqrt(HD)))
        wvs = w_attn[:, 2 * C + h0:2 * C + h0 + HPC * HD]
        per_core.append(dict(
            xT=np.ascontiguousarray(x[b].T).astype(d_b),
            wqk=np.ascontiguousarray(np.concatenate([wq, wk], axis=1)).astype(d_b),
            wv=np.ascontiguousarray(wvs).astype(d_b),
            wo=np.ascontiguousarray(w_proj[h0:h0 + HPC * HD, :]).astype(d_b),
            cost=cos128, ssin=ssin128, utri=utri,
        ))
    return per_core


def kernel(x, w_attn, w_proj):
    from concourse.bass_utils import run_bass_kernel_spmd

    x = np.asarray(x, dtype=np.float32)
    w_attn = np.asarray(w_attn, dtype=np.float32)
    w_proj = np.asarray(w_proj, dtype=np.float32)

    if "nc" not in _CACHE:
        _CACHE["nc"], _CACHE["names"] = _build(T)
    nc, names = _CACHE["nc"], _CACHE["names"]

    per_core = _core_inputs(x, w_attn, w_proj, T)
    in_maps = [{names[k]: v for k, v in m.items()} for m in per_core]
    r = run_bass_kernel_spmd(nc, in_maps, core_ids=list(range(N_CORES)))

    full = np.zeros((B, T, C), dtype=np.float64)
    for core in range(N_CORES):
        full[core // 4] += r.results[core][names["out"]].astype(np.float64)
    return full.astype(np.float32)


# revision 41
# speedup vs baseline: 1.0381x; 1.0232x over previous
"""Causal self-attention (RoPE, 16 heads, B=2 T=2048 C=1024) on 8 TRN2 cores.

Sharding: core = b*4 + g  (b = batch, g = head-group of 4 heads).
Each core computes the qkv projection for its 4 heads, RoPE, causal flash
attention, and the w_proj partial product for its head slice; the host sums
the 4 partials per batch.

v2 design notes (from HW trace analysis):
- The PE clock gate (HAM) halves the PE clock unless it streams
  back-to-back; dependency stalls in the attention loop kept the whole
  attention phase at 1.2 GHz in v1.  v2 keeps the PE fed: qkv work is
  emitted as filler inside pair 0's attention loop and the output
  projection inside pair 1's, one unit per attention iteration.
- Scores use K=64 row-tiled matmul pairs (heads 2hp/2hp+1 on PE rows 0-63
  and 64-127) which execute concurrently on the PE sub-arrays.
- exp() alternates between the Scalar engine (exact table lookup) and the
  Vector engine (Schraudolph bit-trick straight into bf16 bits:
  int16(x*A + B), ~2% rel err) so neither engine paces the loop.
- Attention runs on 512-wide query slices so PSUM fits: score ring 4 banks
  + y ring 2 banks + qkv/proj accumulator ring 2 banks = 8 banks.
"""

import numpy as np

# Problem constants (hardcoded per harness contract).
B = 2
T = 2048
C = 1024
N_HEAD = 16
HD = 64
HPC = 4           # heads per core
N_CORES = 8
ROPE_BASE = 10000.0
TS = 512          # qkv t-slice width
TQ = 512          # attention query-slice width
VW = HD + 1       # v_ext per-head width (v + ones column for denominators)

DTCFG = "bbbb"    # kept for test.py compat; v2 is bf16-only

# Schraudolph exp -> bf16 bits: bf16(exp(x)) ~ int16(x * A + B)
SCH_A = 128.0 / float(np.log(2.0))
SCH_B = 16256.0 - 7.4

_CACHE = {}


def _np_dt(ch):
    if ch == "b":
        import ml_dtypes
        return np.dtype(ml_dtypes.bfloat16)
    return np.dtype(np.float32)


def _build(t_len=T, dtcfg=None, debug=False):
    import concourse.tile as tile
    import concourse.bass as bass
    from concourse import bacc, mybir

    F32 = mybir.dt.float32
    BF16 = mybir.dt.bfloat16
    I16 = mybir.dt.int16
    MULT = mybir.AluOpType.mult
    ADD = mybir.AluOpType.add
    EXP = mybir.ActivationFunctionType.Exp

    n_ts = t_len // TS          # qkv t-slices
    n_tt = t_len // 128         # 128-row t-tiles
    n_j = t_len // TQ           # attention query slices per head pair
    n_ipj = TQ // 128           # new key tiles per query slice

    nc = bacc.Bacc(None, target_bir_lowering=False, debug=False)
    with tile.TileContext(nc) as tc:
        with tc.tile_pool(name="dram", bufs=1, space="DRAM") as dram:
            xT = dram.tile([C, t_len], BF16, kind="ExternalInput")
            wqk = dram.tile([C, 8 * HD], BF16, kind="ExternalInput")
            wv = dram.tile([C, 4 * HD], BF16, kind="ExternalInput")
            wo = dram.tile([4 * HD, C], BF16, kind="ExternalInput")
            cost = dram.tile([128, t_len], BF16, kind="ExternalInput")
            ssin = dram.tile([128, t_len], BF16, kind="ExternalInput")
            utri = dram.tile([128, 128], BF16, kind="ExternalInput")
            out = dram.tile([t_len, C], F32, kind="ExternalOutput")

            xT_c = xT.rearrange("(a p) t -> a p t", p=128)    # [8, 128, T]
            wqk_c = wqk.rearrange("(a p) m -> a p m", p=128)  # [8, 128, 512]
            wv_c = wv.rearrange("(a p) m -> a p m", p=128)    # [8, 128, 256]
            wo_c = wo.rearrange("(a p) m -> a p m", p=128)    # [2, 128, 1024]

            with (
                tc.tile_pool(name="persist", bufs=1) as persist,
                tc.tile_pool(name="rope_pool", bufs=3) as rope_pool,
                tc.tile_pool(name="p_pool", bufs=6) as p_pool,
                tc.tile_pool(name="n_pool", bufs=3) as n_pool,
                tc.tile_pool(name="o_pool", bufs=4) as o_pool,
                tc.tile_pool(name="acc_ps", bufs=2, space="PSUM") as acc_ps,
                tc.tile_pool(name="s_ps", bufs=4, space="PSUM") as s_ps,
                tc.tile_pool(name="y_ps", bufs=2, space="PSUM") as y_ps,
            ):
                # ---- persistent SBUF ----
                utri_sb = persist.tile([128, 128], BF16)
                cos_sb = persist.tile([128, t_len], BF16)
                ssin_sb = persist.tile([128, t_len], BF16)
                wqk_sb = [persist.tile([128, 8 * HD], BF16, name=f"wqk{c}")
                          for c in range(8)]
                wv_sb = [persist.tile([128, 4 * HD], BF16, name=f"wv{c}")
                         for c in range(8)]
                wo_sb = [persist.tile([128, C], BF16, name=f"wo{k}")
                         for k in range(2)]
                xT_sb = [persist.tile([128, t_len], BF16, name=f"xTsb{c}")
                         for c in range(8)]
                qkT = [persist.tile([128, t_len], BF16, name=f"qkT{m}")
                       for m in range(4)]
                vext_sb = persist.tile([128, n_tt * HPC * VW], BF16)
                vext_v = vext_sb.rearrange("p (i h d) -> p i h d",
                                           i=n_tt, d=VW)
                yT = [persist.tile([128, t_len], BF16, name=f"yT{k}")
                      for k in range(2)]

                # ones columns of v_ext (memset; v copies overwrite cols 0-63)
                nc.gpsimd.memset(vext_sb, 1.0)

                # ---- input DMAs ----
                # weights first (small, needed by the first matmuls), then x
                # and the RoPE tables in 512-column waves across 4 queues so
                # wave 0 lands in ~2us and compute starts immediately.  The
                # scalar queue carries no DMA (it is the exp engine).
                qs = [nc.sync, nc.gpsimd, nc.scalar]
                # order: wv + x-wave0 (unblocks the v matmuls at ~4us),
                # then RoPE tables + wqk (first q/k group), then the rest
                # of x as one full-width descriptor per chunk
                for c in range(8):
                    qs[c % 3].dma_start(out=wv_sb[c], in_=wv_c[c])
                for c in range(8):
                    qs[c % 3].dma_start(out=xT_sb[c][:, 0:TS],
                                        in_=xT_c[c, :, 0:TS])
                nc.sync.dma_start(out=cos_sb[:, 0:TS], in_=cost[:, 0:TS])
                nc.gpsimd.dma_start(out=ssin_sb[:, 0:TS],
                                    in_=ssin[:, 0:TS])
                for c in range(8):
                    qs[c % 3].dma_start(out=wqk_sb[c], in_=wqk_c[c])
                nc.scalar.dma_start(out=utri_sb, in_=utri[:])
                nc.sync.dma_start(out=cos_sb[:, TS:], in_=cost[:, TS:])
                nc.gpsimd.dma_start(out=ssin_sb[:, TS:], in_=ssin[:, TS:])
                for c in range(8):
                    qs[c % 3].dma_start(out=xT_sb[c][:, TS:],
                                        in_=xT_c[c, :, TS:])
                for k in range(2):
                    qs[k].dma_start(out=wo_sb[k], in_=wo_c[k])

                # ---- emission units ----
                def emit_v_unit(tt):
                    """v projection for t-tiles tt, tt+1 -> vext."""
                    vps = acc_ps.tile([128, 512], F32, tag="acc",
                                      name=f"vps_{tt}")
                    for b2 in range(2):
                        t0 = (tt + b2) * 128
                        for c in range(8):
                            nc.tensor.matmul(
                                out=vps[:, b2 * 256:(b2 + 1) * 256],
                                lhsT=xT_sb[c][:, t0:t0 + 128],
                                rhs=wv_sb[c][:],
                                start=(c == 0), stop=(c == 7),
                            )
                    nc.scalar.copy(
                        out=vext_v[:, tt:tt + 2, :, :HD],
                        in_=vps.rearrange("p (b h d) -> p b h d", b=2, d=HD),
                    )

                def emit_qk_unit(m, ts):
                    """q/k projection group (m, t-slice ts) + RoPE."""
                    t0 = ts * TS
                    qkps = acc_ps.tile([128, TS], F32, tag="acc",
                                       name=f"qkps_{m}_{ts}")
                    for c in range(8):
                        nc.tensor.matmul(
                            out=qkps[:],
                            lhsT=wqk_sb[c][:, m * 128:(m + 1) * 128],
                            rhs=xT_sb[c][:, t0:t0 + TS],
                            start=(c == 0), stop=(c == 7),
                        )
                    # RoPE: qkT = qkps*cos + blockswap32(qkps*ssin_preswap)
                    u = rope_pool.tile([128, TS], BF16, tag="u",
                                       name=f"u_{m}_{ts}")
                    nc.vector.tensor_mul(u, qkps[:], ssin_sb[:, t0:t0 + TS])
                    u2 = rope_pool.tile([128, TS], BF16, tag="u2",
                                        name=f"u2_{m}_{ts}")
                    for hb in (0, 64):
                        nc.sync.dma_start(out=u2[hb:hb + 32, :],
                                          in_=u[hb + 32:hb + 64, :])
                        nc.sync.dma_start(out=u2[hb + 32:hb + 64, :],
                                          in_=u[hb:hb + 32, :])
                    t1 = rope_pool.tile([128, TS], BF16, tag="t1",
                                        name=f"t1_{m}_{ts}")
                    nc.vector.tensor_mul(t1, qkps[:], cos_sb[:, t0:t0 + TS])
                    nc.gpsimd.tensor_add(qkT[m][:, t0:t0 + TS], t1, u2)

                def emit_proj_unit(tt):
                    """output projection + store for t-tile tt."""
                    ops = [acc_ps.tile([128, 512], F32, tag="acc",
                                       name=f"ops_{tt}_{cs}")
                           for cs in range(2)]
                    for k in range(2):
                        for cs in range(2):
                            nc.tensor.matmul(
                                out=ops[cs][:],
                                lhsT=yT[k][:, tt * 128:(tt + 1) * 128],
                                rhs=wo_sb[k][:, cs * 512:(cs + 1) * 512],
                                start=(k == 0), stop=(k == 1),
                            )
                    for cs in range(2):
                        osb = o_pool.tile([128, 512], F32, tag="osb",
                                          name=f"osb_{tt}_{cs}")
                        if cs == 0:
                            nc.scalar.copy(out=osb, in_=ops[cs][:])
                        else:
                            nc.vector.tensor_copy(out=osb, in_=ops[cs][:])
                        (nc.sync if cs == 0 else nc.gpsimd).dma_start(
                            out=out[tt * 128:(tt + 1) * 128,
                                    cs * 512:(cs + 1) * 512],
                            in_=osb,
                        )

                # ---- attention for one head pair ----
                def attention_pair(hp, filler_by_slice):
                    """Heads (2hp, 2hp+1); consumes one PE filler unit per
                    iteration (per slice j from filler_by_slice[j])."""
                    qtile, ktile = qkT[hp], qkT[2 + hp]
                    heads = (0, 1)

                    for j in range(n_j):
                        filler = list(filler_by_slice.get(j, []))
                        base = TQ * j
                        n_i = n_ipj * j + n_ipj
                        # burst 2 filler units/iter at the slice start (covers
                        # the PE while the previous slice's exp backlog
                        # drains), then spread the rest over the slice
                        fire = {}
                        if filler:
                            nb = min(len(filler), 4)
                            fire = {0: 2, 1: 2} if nb >= 4 else {0: nb}
                            rest = len(filler) - sum(fire.values())
                            if rest > 0:
                                step = max(1, (n_i - 2) // rest)
                                for k in range(rest):
                                    it = min(2 + k * step, n_i - 1)
                                    fire[it] = fire.get(it, 0) + 1
                        yps = {hh: y_ps.tile([VW, TQ], F32, tag="yps",
                                             name=f"yps_{hp}_{hh}_{j}")
                               for hh in heads}
                        pend = []

                        def emit_s(i):
                            off = max(base, 128 * i) - base
                            sx = {}
                            for hh in heads:
                                sx[hh] = s_ps.tile([128, TQ], F32, tag="sps",
                                                   name=f"s_{hp}_{hh}_{j}_{i}")
                            for hh in heads:
                                hoff = 64 * hh
                                nc.tensor.matmul(
                                    out=sx[hh][:, off:],
                                    lhsT=ktile[hoff:hoff + 64,
                                               128 * i:128 * (i + 1)],
                                    rhs=qtile[hoff:hoff + 64,
                                              base + off:base + TQ],
                                    start=True, stop=True,
                                )
                            px = p_pool.tile([128, 2 * TQ], BF16, tag="px",
                                             name=f"px_{hp}_{j}_{i}")
                            for hh in heads:
                                hcol = TQ * hh
                                if (i + hh) % 2 == 0:
                                    nc.scalar.activation(
                                        out=px[:, hcol + off:hcol + TQ],
                                        in_=sx[hh][:, off:], func=EXP)
                                else:
                                    e16 = px.bitcast(I16)
                                    nc.vector.tensor_scalar(
                                        out=e16[:, hcol + off:hcol + TQ],
                                        in0=sx[hh][:, off:],
                                        scalar1=SCH_A, scalar2=SCH_B,
                                        op0=MULT, op1=ADD)
                            if i >= n_ipj * j:
                                for hh in heads:
                                    hcol = TQ * hh
                                    nc.vector.tensor_mul(
                                        px[:, hcol + off:hcol + off + 128],
                                        px[:, hcol + off:hcol + off + 128],
                                        utri_sb)
                            pend.append((i, px))

                        def emit_y():
                            i, px = pend.pop(0)
                            off = max(base, 128 * i) - base
                            for hh in heads:
                                gh = 2 * hp + hh
                                base_v = (i * HPC + gh) * VW
                                nc.tensor.matmul(
                                    out=yps[hh][:, off:],
                                    lhsT=vext_sb[:, base_v:base_v + VW],
                                    rhs=px[:, TQ * hh + off:TQ * (hh + 1)],
                                    start=(i == 0), stop=(i == n_i - 1),
                                    skip_group_check=True,
                                )

                        for i in range(n_i):
                            emit_s(i)
                            for _ in range(fire.get(i, 0)):
                                if filler:
                                    filler.pop(0)()
                            if i >= 2:
                                emit_y()
                        emit_y()
                        emit_y()
                        for f in filler:   # leftovers (shouldn't happen)
                            f()

                        # normalization: y /= softmax denominator (row 64)
                        ycp = n_pool.tile([VW, 2 * TQ], F32, tag="ycp",
                                          name=f"ycp_{hp}_{j}")
                        nc.scalar.copy(out=ycp[:, 0:TQ], in_=yps[0][:])
                        nc.scalar.copy(out=ycp[:, TQ:], in_=yps[1][:])
                        strip = n_pool.tile([8, 128], F32, tag="strip",
                                            name=f"strip_{hp}_{j}")
                        nc.sync.dma_start(
                            out=strip,
                            in_=ycp[HD:HD + 1, :].rearrange(
                                "p (a b) -> p a b", b=128))
                        rstrip = n_pool.tile([8, 128], F32, tag="rstrip",
                                             name=f"rstrip_{hp}_{j}")
                        nc.vector.reciprocal_approx_fast(out=rstrip,
                                                         in_=strip)
                        rrow = n_pool.tile([1, 2 * TQ], F32, tag="rrow",
                                           name=f"rrow_{hp}_{j}")
                        nc.sync.dma_start(
                            out=rrow.rearrange("p (a b) -> p a b", b=128),
                            in_=rstrip)
                        bcast = n_pool.tile([64, 2 * TQ], F32, tag="bcast",
                                            name=f"bcast_{hp}_{j}")
                        nc.gpsimd.partition_broadcast(bcast[:], rrow[:])
                        for hh in heads:
                            nc.vector.tensor_mul(
                                yT[hp][64 * hh:64 * hh + 64,
                                       base:base + TQ],
                                ycp[0:HD, TQ * hh:TQ * hh + TQ],
                                bcast[:, TQ * hh:TQ * hh + TQ])

                # ---- schedule ----
                # upfront: minimum for pair-0 slice 0 (v tiles 0-1, q/k ts0)
                emit_v_unit(0)
                emit_qk_unit(2, 0)   # k01 ts0
                emit_qk_unit(0, 0)   # q01 ts0

                # pair 0 attention; filler = rest of qkv, scheduled so each
                # slice's q/k and v dependencies are emitted a slice ahead
                fill0 = {
                    0: [lambda: emit_v_unit(2),
                        lambda: emit_qk_unit(2, 1),
                        lambda: emit_qk_unit(0, 1),
                        lambda: emit_v_unit(4)],
                    1: [lambda: emit_v_unit(6),
                        lambda: emit_qk_unit(2, 2),
                        lambda: emit_qk_unit(0, 2),
                        lambda: emit_v_unit(8),
                        lambda: emit_v_unit(10)],
                    2: [lambda: emit_qk_unit(2, 3),
                        lambda: emit_qk_unit(0, 3),
                        lambda: emit_v_unit(12),
                        lambda: emit_v_unit(14),
                        lambda: emit_qk_unit(3, 0),
                        lambda: emit_qk_unit(1, 0)],
                    3: [lambda: emit_qk_unit(3, 1),
                        lambda: emit_qk_unit(1, 1),
                        lambda: emit_qk_unit(3, 2),
                        lambda: emit_qk_unit(1, 2),
                        lambda: emit_qk_unit(3, 3),
                        lambda: emit_qk_unit(1, 3)],
                }
                attention_pair(0, fill0)

                # pair 1 attention; filler = output projection for t-tiles
                # whose yT rows are complete (tile tt needs pair-1 slice
                # tt//n_ipj normed; norms for slice j are emitted at the
                # end of slice j)
                fill1 = {
                    1: [lambda tt=tt: emit_proj_unit(tt) for tt in range(2)],
                    2: [lambda tt=tt: emit_proj_unit(tt)
                        for tt in range(2, 6)],
                    3: [lambda tt=tt: emit_proj_unit(tt)
                        for tt in range(6, 12)],
                }
                attention_pair(1, fill1)

                # tail: remaining projection tiles
                for tt in range(12, n_tt):
                    emit_proj_unit(tt)

    nc.compile()
    names = dict(
        xT=xT.name, wqk=wqk.name, wv=wv.name, wo=wo.name,
        cost=cost.name, ssin=ssin.name, utri=utri.name,
        out=out.name,
    )
    return nc, names


# Head-dim permutation: evens first, odds last — turns the interleaved
# rotate-half pair swap into a contiguous 32-row block swap on device.
PERM = np.concatenate([np.arange(0, HD, 2), np.arange(1, HD, 2)])


def _host_constants(t_len=T, dtcfg=None):
    inv_freq = 1.0 / (ROPE_BASE ** (np.arange(0, HD, 2, dtype=np.float64) / HD))
    t = np.arange(t_len, dtype=np.float64)
    freqs = np.outer(t, inv_freq)                      # [T, 32]
    emb = np.concatenate([freqs, freqs], axis=-1)      # [T, 64]
    cosT = np.cos(emb).T.astype(np.float32)            # [64, T]
    sinT = np.sin(emb).T.astype(np.float32)
    sgn = np.where(np.arange(HD) % 2 == 0, -1.0, 1.0).astype(np.float32)
    ssinT = sinT * sgn[:, None]
    cosP, ssinP = cosT[PERM], ssinT[PERM]
    d_b = _np_dt("b")
    cos128 = np.vstack([cosP, cosP]).astype(d_b)       # [128, T]
    ssin128 = np.vstack([ssinP, ssinP])
    # pre-swap the sin table in 32-blocks: on device the product
    # u = q*ssin_preswap is block-swapped, giving swap(q)*ssin
    sw = np.concatenate([np.arange(32, 64), np.arange(0, 32),
                         np.arange(96, 128), np.arange(64, 96)])
    ssin128 = ssin128[sw].astype(d_b)
    utri = np.triu(np.ones((128, 128), dtype=np.float32)).astype(d_b)
    return cos128, ssin128, utri


def _perm_heads(w):
    """Permute each head's 64 columns of w [C, HPC*HD] by PERM."""
    Cdim = w.shape[0]
    return w.reshape(Cdim, HPC, HD)[:, :, PERM].reshape(Cdim, HPC * HD)


def _core_inputs(x, w_attn, w_proj, t_len=T, dtcfg=None):
    """Build the per-core input maps (values only, keyed by logical name)."""
    d_b = _np_dt("b")
    cos128, ssin128, utri = _host_constants(t_len, dtcfg)
    per_core = []
    for core in range(N_CORES):
        b, g = divmod(core, 4)
        h0 = g * HPC * HD                       # column offset of first head
        wq = _perm_heads(w_attn[:, h0:h0 + HPC * HD])
        wk = _perm_heads(w_attn[:, C + h0:C + h0 + HPC * HD]
                         * np.float32(1.0 / np.sqrt(HD)))
        wvs = w_attn[:, 2 * C + h0:2 * C + h0 + HPC * HD]
        per_core.append(dict(
            xT=np.ascontiguousarray(x[b].T).astype(d_b),
            wqk=np.ascontiguousarray(np.concatenate([wq, wk], axis=1)).astype(d_b),
            wv=np.ascontiguousarray(wvs).astype(d_b),
            wo=np.ascontiguousarray(w_proj[h0:h0 + HPC * HD, :]).astype(d_b),
            cost=cos128, ssin=ssin128, utri=utri,
        ))
    return per_core


def kernel(x, w_attn, w_proj):
    from concourse.bass_utils import run_bass_kernel_spmd

    x = np.asarray(x, dtype=np.float32)
    w_attn = np.asarray(w_attn, dtype=np.float32)
    w_proj = np.asarray(w_proj, dtype=np.float32)

    if "nc" not in _CACHE:
        _CACHE["nc"], _CACHE["names"] = _build(T)
    nc, names = _CACHE["nc"], _CACHE["names"]

    per_core = _core_inputs(x, w_attn, w_proj, T)
    in_maps = [{names[k]: v for k, v in m.items()} for m in per_core]
    r = run_bass_kernel_spmd(nc, in_maps, core_ids=list(range(N_CORES)))

    full = np.zeros((B, T, C), dtype=np.float64)
    for core in range(N_CORES):
        full[core // 4] += r.results[core][names["out"]].astype(np.float64)
    return full.astype(np.float32)
